# revision 18
# baseline (speedup 1.0000x reference)
"""Multi-head attention kernel for Trainium2, 8 NeuronCores.

Problem: B=4, T=2048, D_in=1024, 16 heads x 64 dim, E=1024 (fp32).

Sharding: (batch x head-group). Core c handles batch b=c//2 and head-group
g=c%2 (8 heads, 512 qk/v dims). Each core computes its batch's QKV
projections restricted to its heads, full attention for those heads, and a
partial output projection. The host sums the two partial projections per
batch (the only cross-core reduction) and stacks batches.

Per-core dataflow (matmuls bf16 inputs, fp32 PSUM accumulation):
  xT      = dma-xbar-transpose(cast_bf16(x))            [1024, 2048] per tensor
  qhT/khT = w.T @ xT   (weights stationary)             [512, 2048]
  vh      = xT.T @ wv  (x stationary) + ones column     [2048, 8*65]
  S^T     = khT_h.T @ qhT_h per head pair               PSUM [128,1024]
  expS    = ACT exp(S^T/8) -> bf16 SBUF                 (the softmax exp)
  AV      = es_chunk.T @ vh_ext  ([q,d] orientation)    PSUM [128 q, 65]
            col 64 = softmax denominator (ones column); accumulated over
            key tiles with start=False into DVE-zeroed PSUM banks
  attnU   = (AV[:,0:64] * recip(AV[:,64])) -> bf16      normalize on evac
  attnT   = PE-transpose(attnU)                         [2h*64, q] per pair
  y      += attnT_m.T @ wp_m  (K=128 contraction x4)    [2048, 1024] fp32
"""

import sys

import numpy as np

if "/opt/trn_rl_repo" not in sys.path:
    sys.path.insert(0, "/opt/trn_rl_repo")

B, T, DIN = 4, 2048, 1024
NH, HD, EMB = 16, 64, 1024
HGD = 512          # per-core qk/v dims (8 heads * 64)
NKT = DIN // 128   # 8  input-dim k tiles
NQC = T // 512     # 4  t chunks of 512
NTT = T // 128     # 16 t tiles of 128
NM = HGD // 128    # 4  head-pair m tiles
HPC = 8            # heads per core

_CACHE = {}

# build-time tunables
TUNE = {"ES_BUFS": 38, "CAP": 64, "WARMUP": 52, "PS_BUFS": 2, "BIG_BUFS": 2,
        "DEFER_EVERY": 2, "AV_LIMIT": 2, "LOOKW": 3, "LOOKW2": 2, "CAP2": 30,
        "XT_BUFS": 3}


def _build_nc():
    import concourse.bacc as bacc
    import concourse.bass as bass
    import concourse.mybir as mybir
    import concourse.tile as tile

    dt = mybir.dt
    AF = mybir.ActivationFunctionType

    nc = bacc.Bacc("TRN2", target_bir_lowering=False, debug=False)
    # inputs arrive host-cast to bf16 (the exact truncation the device-side
    # DMA cast performed) and weights host-rearranged to the k-tiled SBUF
    # layouts, so every load is a plain HWDGE DMA with no SWDGE ring.
    xq = nc.declare_dram_parameter("xq", [T, DIN], dt.bfloat16, isOutput=False)
    xk = nc.declare_dram_parameter("xk", [T, DIN], dt.bfloat16, isOutput=False)
    xv = nc.declare_dram_parameter("xv", [T, DIN], dt.bfloat16, isOutput=False)
    # wkq = [wk | wq] k-tiled+flattened, wvi = [wv | ident]: packed so the
    # critical-path weight loads are 2 large DMAs (the tile scheduler chains
    # DMAs one-after-another, so each extra DMA costs ~2us of dead time)
    wkq = nc.declare_dram_parameter("wkq", [128, 2 * NKT * HGD], dt.bfloat16, isOutput=False)
    wvi = nc.declare_dram_parameter("wvi", [128, NKT * HGD + 128], dt.bfloat16, isOutput=False)
    wp = nc.declare_dram_parameter("wp", [128, NM, EMB], dt.bfloat16, isOutput=False)
    y = nc.declare_dram_parameter("y", [T, EMB], dt.float32, isOutput=True)

    with tile.TileContext(nc) as tc:
        from contextlib import ExitStack

        with ExitStack() as ctx:
            p_w = ctx.enter_context(tc.tile_pool(name="weights", bufs=1))
            p_xt = ctx.enter_context(tc.tile_pool(name="xt", bufs=TUNE["XT_BUFS"]))
            p_qkh = ctx.enter_context(tc.tile_pool(name="qkh", bufs=1))
            p_vh = ctx.enter_context(tc.tile_pool(name="vh", bufs=1))
            p_exps = ctx.enter_context(tc.tile_pool(name="exps", bufs=TUNE["ES_BUFS"]))
            p_attn = ctx.enter_context(tc.tile_pool(name="attn", bufs=1))
            p_au = ctx.enter_context(tc.tile_pool(name="au", bufs=8))
            p_rc = ctx.enter_context(tc.tile_pool(name="rc", bufs=8))
            p_y = ctx.enter_context(tc.tile_pool(name="ysb", bufs=3))
            p_ps = ctx.enter_context(tc.tile_pool(name="psum_s", bufs=TUNE["PS_BUFS"], space="PSUM"))
            p_av = ctx.enter_context(tc.tile_pool(name="psum_av", bufs=1, space="PSUM"))
            p_big = ctx.enter_context(tc.tile_pool(name="psum_big", bufs=TUNE["BIG_BUFS"], space="PSUM"))

            # --- persistent SBUF ---
            WSZ = NKT * HGD
            M0 = NKT * 128  # one m-tile's worth of k-tiled columns
            w_all = p_w.tile([128, 3 * WSZ + 128], dt.bfloat16, tag="wall")
            # layout: [wk m0 | wq m0 | wk m1-3 | wq m1-3 | wv | ident] so the
            # first (small) DMA unlocks pk(b0,0)+pq(b0,0) asap
            wk_m0 = w_all[:, 0:M0].rearrange("p (kt n) -> p kt n", kt=NKT)
            wq_m0 = w_all[:, M0 : 2 * M0].rearrange("p (kt n) -> p kt n", kt=NKT)
            wk_r = w_all[:, 2 * M0 : 2 * M0 + 3 * M0].rearrange(
                "p (kt n) -> p kt n", kt=NKT
            )
            wq_r = w_all[:, 2 * M0 + 3 * M0 : 2 * M0 + 6 * M0].rearrange(
                "p (kt n) -> p kt n", kt=NKT
            )
            wv_sb = w_all[:, 2 * WSZ : 3 * WSZ].rearrange("p (kt n) -> p kt n", kt=NKT)
            id_sb = w_all[:, 3 * WSZ : 3 * WSZ + 128]

            def wk_sl(m, kt):
                return wk_m0[:, kt, :] if m == 0 else wk_r[:, kt, 128 * (m - 1) : 128 * m]

            def wq_sl(m, kt):
                return wq_m0[:, kt, :] if m == 0 else wq_r[:, kt, 128 * (m - 1) : 128 * m]
            wp_sb = p_w.tile([128, NM, EMB], dt.bfloat16, tag="wp")
            wz = p_w.tile([128, 512], dt.bfloat16, tag="wz")

            # qh/kh in fp8e4m3, DoubleRow layout per pair: head h on
            # partitions 32h:32h+32, contraction row (p, i) = dim 32i + p.
            # The S matmuls then run in DoubleRow perf mode (0.5 cycles/row).
            qh8 = [p_qkh.tile([64, 2, T], dt.float8e4, tag=f"qh8{m}", name=f"qh8{m}") for m in range(NM)]
            kh8 = [p_qkh.tile([64, 2, T], dt.float8e4, tag=f"kh8{m}", name=f"kh8{m}") for m in range(NM)]
            # vh_ext[t, h, 0:64] = vh, vh_ext[t, h, 64] = 1.0 (softmax denom)
            vh_ext = [p_vh.tile([128, HPC, HD + 1], dt.bfloat16, tag=f"vh{tt}", name=f"vh{tt}") for tt in range(NTT)]
            nc.vector.memset(wz[:], 0.0)
            for tt in range(NTT):
                nc.vector.memset(vh_ext[tt][:, :, HD : HD + 1], 1.0)
            # attnT[m]: head 2m in rows 0:64, head 2m+1 in rows 64:128
            attnT = [p_attn.tile([128, T], dt.bfloat16, tag=f"at{m}", name=f"at{m}") for m in range(NM)]

            # --- PE warmup: keeps the tensor engine busy (and its p-state
            # ramping to full clock) while the first input DMAs land ---
            for i in range(TUNE["WARMUP"]):
                pw = p_big.tile([128, 512], dt.float32, tag="psb", name="pwz")
                nc.tensor.matmul(pw[:], wz[:, 0:128], wz[:], start=True, stop=True)

            # --- DMA program ---
            # Weights on the ACT HWDGE queue (issued up-front, no data
            # waits); xbar transposes + wp + y on the sync queue. Transpose
            # order: K blocks (gate the S/exp stream) and q0 first, V blocks
            # (gate AV drains = es recycling) interleaved, q1-q3 trail.
            nc.scalar.dma_start(out=w_all[:, 0 : 2 * M0], in_=wkq[:, 0 : 2 * M0])

            def load_T(xb, qcb):
                """xbar-transpose one 512-token block of a bf16 input param.

                xt[p, kt, t] = x[512*qcb + t, kt*128 + p]

                The XPOSE ISA instruction has a single semaphore-wait slot;
                the input params need no RAW wait, so even reused pool slots
                carry only the WAR wait.
                """
                xt = p_xt.tile([128, NKT, 512], dt.bfloat16, tag="xt")
                nc.sync.dma_start(
                    out=xt[:], in_=xb[512 * qcb : 512 * (qcb + 1), :], transpose=True
                )
                return xt

            # ---- attention state machine ----
            # Window = (qc, pair): 2 heads x 512 queries x all 2048 keys.
            # exactly one window owns the two AV PSUM banks at a time; later
            # windows run S+exp ahead into SBUF es slots (lookahead bounded by
            # the exps pool) so the scalar engine never idles.
            windows = [(qc, pair) for qc in range(NQC) for pair in range(NM)]
            NW = len(windows)
            sdone = {w: 0 for w in windows}   # exps emitted
            adone = {w: 0 for w in windows}   # AV key-tiles drained
            buf = {w: [] for w in windows}    # (kt, es) pending AV
            av_cur = {}                       # owner window -> (av_a, av_b)
            st = {"owner": 0, "inflight": 0, "ktv": 0, "qav": 0,
                  "credit": 0, "heavy_at": 0, "drain_at": 0, "fin_at": 0,
                  "cap": TUNE["CAP"], "lookw": TUNE["LOOKW"]}
            ktk = [0] * NM   # per-pair key-tile availability
            kbl = [set() for _ in range(NM)]  # emitted k-blocks per pair
            qavm = [0] * NM  # per-pair contiguous q-block availability

            def bump_k(m, b):
                kbl[m].add(b)
                n = 0
                while n in kbl[m]:
                    n += 1
                ktk[m] = 4 * n

            DEFER_EVERY = TUNE["DEFER_EVERY"]
            defer_q = []
            fin_q = []
            CAP = TUNE["CAP"]
            proj_pend = []                    # pending proj groups (closures)

            def emit_s_exp(w):
                qc, pair = w
                kt = sdone[w]
                qsl_w = slice(512 * qc, 512 * (qc + 1))
                ksl = slice(128 * kt, 128 * (kt + 1))
                ps = p_ps.tile([128, 1024], dt.float32, tag="pss", name="pss")
                for h in range(2):
                    nc.tensor.matmul(
                        ps[:, 512 * h : 512 * h + 512],
                        kh8[pair][32 * h : 32 * h + 32, :, ksl],
                        qh8[pair][32 * h : 32 * h + 32, :, qsl_w],
                        start=True, stop=True,
                        perf_mode=mybir.MatmulPerfMode.DoubleRow,
                    )
                es = p_exps.tile([128, 1024], dt.bfloat16, tag="es", name="es")
                nc.scalar.activation(es[:], ps[:], AF.Exp, scale=1.0 / 8.0)
                buf[w].append((kt, es))
                sdone[w] += 1
                st["inflight"] += 1

            def open_av(w):
                # no memset: the window's first drained kt writes every
                # (h, qs) region with start=True, which zeroes it in-psum
                av_a = p_av.tile([128, NQC * (HD + 1)], dt.float32, tag="ava", name="ava")
                av_b = p_av.tile([128, NQC * (HD + 1)], dt.float32, tag="avb", name="avb")
                av_cur[w] = (av_a, av_b)

            def emit_av(w, limit=TUNE["AV_LIMIT"]):
                """Drain up to `limit` buffered (kt, es) pairs whose vh tiles
                have landed. Bounded so a backlogged window's drains spread
                between S emissions instead of wedging 3.5us of PE work
                between consecutive exps."""
                qc, pair = w
                if w not in av_cur:
                    open_av(w)
                av = av_cur[w]
                rest = []
                done = 0
                for kt, es in buf[w]:
                    if kt >= st["ktv"] or done >= limit:
                        rest.append((kt, es))
                        continue
                    first = adone[w] == 0 and done == 0
                    done += 1
                    for h in range(2):
                        for qs in range(4):
                            nc.tensor.matmul(
                                av[h][:, 65 * qs : 65 * qs + 65],
                                es[:, 512 * h + 128 * qs : 512 * h + 128 * qs + 128],
                                vh_ext[kt][:, 2 * pair + h, :],
                                start=first, stop=True, skip_group_check=True,
                            )
                    adone[w] += 1
                    st["inflight"] -= 1
                drained = len(buf[w]) - len(rest)
                buf[w][:] = rest
                return drained

            def close_window(w):
                """Normalize the finished window (DVE); the PE transposes are
                deferred a couple of exps so they don't park in the PE wait
                queue (depth 4) ahead of the next S matmuls."""
                qc, pair = w
                av = av_cur.pop(w)
                # one strided reciprocal per head over the 4 denominator
                # columns (fewer DVE instructions waiting on AV sems)
                rcs = []
                for h in range(2):
                    rc = p_rc.tile([128, 4], dt.float32, tag="rc", name="rc")
                    nc.vector.reciprocal(
                        rc[:], av[h].rearrange("p (qs c) -> p qs c", qs=4)[:, :, 64]
                    )
                    rcs.append(rc)
                aus = []
                for qs in range(4):
                    au = p_au.tile([128, 128], dt.bfloat16, tag="au", name="au")
                    aus.append(au)
                    for h in range(2):
                        nc.vector.tensor_scalar_mul(
                            au[:, 64 * h : 64 * h + 64],
                            av[h][:, 65 * qs : 65 * qs + 64],
                            rcs[h][:, qs : qs + 1],
                        )

                def fin():
                    # rides an S-psum slot: fins are ~1 per 16 exps, the
                    # occasional S-slot rotation hiccup is cheap
                    pt = p_ps.tile([128, 512], dt.bfloat16, tag="pss", name="ptt")
                    for qs in range(4):
                        nc.tensor.transpose(
                            pt[:, 128 * qs : 128 * (qs + 1)], aus[qs][:], id_sb[:]
                        )
                    nc.vector.tensor_copy(
                        attnT[pair][:, 512 * qc : 512 * (qc + 1)], pt[:]
                    )
                    if pair == NM - 1:
                        queue_proj(qc)

                fin_q.append(fin)

            def queue_proj(qc):
                for tt in range(4 * qc, 4 * qc + 4):
                    for ec in range(2):
                        queue_proj_halves(tt, ec)

            def queue_proj_halves(tt, ec):
                """Two paced halves per proj group so at most ~2 matmuls of
                proj work lands between consecutive S emissions."""
                tsl = slice(128 * tt, 128 * (tt + 1))
                esl = slice(512 * ec, 512 * (ec + 1))
                d = {}

                def half_a():
                    # past the last exp the S-psum banks are idle: alternate
                    # between them and psb so tail proj groups double-buffer
                    if st["owner"] >= NW and (tt + ec) % 2 == 0:
                        d["ps"] = p_ps.tile([128, 512], dt.float32, tag="pss", name="pssp")
                    else:
                        d["ps"] = p_big.tile([128, 512], dt.float32, tag="psb", name="psb")
                    for m in range(2):
                        nc.tensor.matmul(
                            d["ps"][:], attnT[m][:, tsl], wp_sb[:, m, esl],
                            start=(m == 0), stop=False,
                        )

                def half_b():
                    ps = d["ps"]
                    for m in range(2, NM):
                        nc.tensor.matmul(
                            ps[:], attnT[m][:, tsl], wp_sb[:, m, esl],
                            start=False, stop=(m == NM - 1), skip_group_check=True,
                        )
                    ysb = p_y.tile([128, 512], dt.float32, tag="ysb", name="ysb")
                    nc.vector.tensor_copy(ysb[:], ps[:])
                    nc.sync.dma_start(out=y[tsl, esl], in_=ysb[:])

                proj_pend.append(half_a)
                proj_pend.append(half_b)

            def emit_proj_group():
                proj_pend.pop(0)()

            def eligible(w):
                qc, pair = w
                return qc < qavm[pair] and sdone[w] < min(ktk[pair], NTT)

            def progress(exp_budget=10**9):
                """Advance the attention pipeline.

                One S+exp per iteration, round-robin across the owner window
                and the next LOOK_WINS windows (kt-major, so a late K block
                never stalls the whole stream). Heavy deferred work (proj
                groups, deferred pq groups) is credit-paced at one group per
                DEFER_EVERY exps to fill PE slack without starving ACT.
                """
                n = 0
                while True:
                    moved = False
                    # 1. one S+exp first, so it lands ahead of any parked
                    # work. Owner-first keeps AV drains hot on the exp
                    # stream's heels; kt-major round-robin across the
                    # lookahead windows covers ktk-gated stretches.
                    if n < exp_budget and st["owner"] < NW:
                        wo = windows[st["owner"]]
                        if eligible(wo):
                            w2 = wo  # owner never blocked by the cap
                        else:
                            cands = [
                                windows[i]
                                for i in range(
                                    st["owner"] + 1,
                                    min(st["owner"] + 1 + st["lookw"], NW),
                                )
                            ]
                            cands = [x for x in cands if eligible(x)]
                            w2 = (
                                min(cands, key=lambda x: sdone[x])
                                if cands and st["inflight"] < st["cap"]
                                else None
                            )
                        if w2 is not None:
                            emit_s_exp(w2)
                            st["credit"] += 1
                            n += 1
                            moved = True
                    # 2. deferred close transposes (after >=2 exps of spacing)
                    if fin_q and st["credit"] >= st["fin_at"]:
                        fin_q.pop(0)()
                        moved = True
                    # 3. owner AV drains (cooldown after an ownership change
                    #    so drains don't park behind the memset chain)
                    if st["owner"] < NW:
                        w = windows[st["owner"]]
                        if buf[w] and st["credit"] >= st["drain_at"]:
                            emit_av(w)
                        if sdone[w] == NTT and adone[w] == NTT:
                            close_window(w)
                            st["owner"] += 1
                            st["drain_at"] = st["credit"] + 3
                            st["fin_at"] = st["credit"] + 3
                            moved = True
                    # 4. paced heavy work: deferred pq first (unlocks windows),
                    #    then proj groups
                    if st["credit"] >= st["heavy_at"] and (proj_pend or defer_q):
                        if defer_q:
                            defer_q.pop(0)()
                        else:
                            emit_proj_group()
                        st["heavy_at"] = st["credit"] + DEFER_EVERY
                        moved = True
                    if not moved:
                        # nothing credit-eligible moved: force one pending
                        # action so the pacing counters can never deadlock
                        if (
                            st["owner"] < NW
                            and buf[windows[st["owner"]]]
                            and emit_av(windows[st["owner"]])
                        ):
                            moved = True
                        elif fin_q:
                            fin_q.pop(0)()
                            moved = True
                        elif defer_q:
                            defer_q.pop(0)()
                            moved = True
                        elif proj_pend:
                            emit_proj_group()
                            moved = True
                    if not moved or n >= exp_budget:
                        break
                if st["owner"] >= NW:
                    while fin_q:
                        fin_q.pop(0)()
                    while defer_q:
                        defer_q.pop(0)()
                    while proj_pend:
                        emit_proj_group()

            # --- QKV projection groups ---
            # each emitted as two ~4-matmul halves so at most half a group
            # of heavy PE work lands between consecutive S emissions
            def _pkq_halves(w_sl, dst, xT, qcb, m):
                qsl = slice(512 * qcb, 512 * (qcb + 1))
                d = {}

                def half_a():
                    d["ps"] = p_big.tile([128, 512], dt.float32, tag="psb", name="psb")
                    for kt in range(4):
                        nc.tensor.matmul(
                            d["ps"][:], w_sl(m, kt), xT[:, kt, :],
                            start=(kt == 0), stop=False,
                        )

                def half_b():
                    ps = d["ps"]
                    for kt in range(4, NKT):
                        nc.tensor.matmul(
                            ps[:], w_sl(m, kt), xT[:, kt, :],
                            start=False, stop=(kt == NKT - 1), skip_group_check=True,
                        )
                    # host permuted this m-tile's weight columns to
                    # [h0 d0:32 | h1 d0:32 | h0 d32:64 | h1 d32:64], so the
                    # DoubleRow fp8 layout evacuates in two contiguous copies
                    for i in range(2):
                        nc.vector.tensor_copy(
                            dst[m][:, i, qsl], ps[64 * i : 64 * i + 64, :]
                        )

                return half_a, half_b

            def pk_halves(xkT, qcb, m):
                return _pkq_halves(wk_sl, kh8, xkT, qcb, m)

            def pq_halves(xqT, qcb, m):
                return _pkq_halves(wq_sl, qh8, xqT, qcb, m)

            def pk(xkT, qcb, m):
                a, b = pk_halves(xkT, qcb, m)
                a()
                b()

            def pq(xqT, qcb, m):
                a, b = pq_halves(xqT, qcb, m)
                a()
                b()

            def pv_halves(xvT, qcb, ti):
                tt = 4 * qcb + ti
                d = {}

                def half_a():
                    d["ps"] = p_big.tile([128, 512], dt.float32, tag="psb", name="psb")
                    for kt in range(4):
                        nc.tensor.matmul(
                            d["ps"][:], xvT[:, kt, 128 * ti : 128 * (ti + 1)],
                            wv_sb[:, kt, :],
                            start=(kt == 0), stop=False,
                        )

                def half_b():
                    ps = d["ps"]
                    for kt in range(4, NKT):
                        nc.tensor.matmul(
                            ps[:], xvT[:, kt, 128 * ti : 128 * (ti + 1)],
                            wv_sb[:, kt, :],
                            start=False, stop=(kt == NKT - 1), skip_group_check=True,
                        )
                    nc.vector.tensor_copy(
                        vh_ext[tt][:, :, 0:HD], ps.rearrange("p (h d) -> p h d", h=HPC)
                    )

                return half_a, half_b

            # --- phase 1 emission: K0+Q0 first (earliest possible S/exp),
            # then K blocks with V interleaved one block behind (es-slot
            # recycling tracks the exp stream), q1 mid, q2-q3 + wp trail. ---
            xkT0 = load_T(xk, 0)
            xqT0 = load_T(xq, 0)
            # big weight chunks ride the sync queue at pinned chain positions
            nc.sync.dma_start(out=w_all[:, 2 * M0 : 2 * WSZ], in_=wkq[:, 2 * M0 :])
            pk(xkT0, 0, 0)
            pq(xqT0, 0, 0)
            bump_k(0, 0)
            qavm[0] = 1
            progress(4)

            def mk_b0(m):
                ka, kb = pk_halves(xkT0, 0, m)
                qa, qb = pq_halves(xqT0, 0, m)

                def kb2():
                    kb()
                    bump_k(m, 0)

                def qb2():
                    qb()
                    qavm[m] = 1

                return [ka, kb2, qa, qb2]

            for m in range(1, NM):
                defer_q.extend(mk_b0(m))
            progress(2)
            progress(2)

            def k_block(b):
                xkT = load_T(xk, b)
                for m in range(NM):
                    a, bb = pk_halves(xkT, b, m)
                    a()
                    progress(1)
                    bb()
                    bump_k(m, b)
                    progress(2)

            def v_block(b, defer=False):
                xvT = load_T(xv, b)

                def mk(ti):
                    a, bb = pv_halves(xvT, b, ti)

                    def done():
                        bb()
                        st["ktv"] = max(st["ktv"], 4 * b + ti + 1)

                    return a, done

                for ti in range(4):
                    a, done = mk(ti)
                    if defer:
                        defer_q.append(a)
                        defer_q.append(done)
                    else:
                        a()
                        progress(1)
                        done()
                        progress(1)

            def q_defer(b):
                # the pq groups for qc b are only needed once ACT reaches
                # qc b's windows -- push them into the paced deferral queue
                xqT = load_T(xq, b)

                def mk(m):
                    a, bb = pq_halves(xqT, b, m)

                    def done():
                        bb()
                        qavm[m] = b + 1

                    return a, done

                for m in range(NM):
                    a, done = mk(m)
                    defer_q.append(a)
                    defer_q.append(done)

            k_block(1)
            nc.sync.dma_start(out=w_all[:, 2 * WSZ : 3 * WSZ + 128], in_=wvi[:, :])
            k_block(2)
            v_block(0)
            k_block(3)
            st["lookw"] = TUNE["LOOKW2"]
            v_block(1)
            v_block(2, defer=True)
            v_block(3, defer=True)
            st["cap"] = TUNE["CAP2"]
            q_defer(1)
            nc.sync.dma_start(out=wp_sb[:, 0:2, :], in_=wp[:, 0:2, :])
            q_defer(2)
            nc.sync.dma_start(out=wp_sb[:, 2:4, :], in_=wp[:, 2:4, :])
            q_defer(3)

            # drain everything
            progress()


    nc.compile()
    return nc


def _get_nc():
    if "nc" not in _CACHE:
        _CACHE["nc"] = _build_nc()
    return _CACHE["nc"]


def _w_ktiled(w, dr_perm=False):
    # [DIN, n] -> [128, DIN//128, n]: w_ktiled[p, kt, :] = w[kt*128 + p, :]
    w = np.asarray(w)
    if dr_perm:
        # permute each 128-col m-tile to [h0 d0:32 | h1 d0:32 | h0 d32:64 |
        # h1 d32:64] so the fp8 DoubleRow evac is 2 contiguous copies
        P = np.concatenate([np.r_[0:32], np.r_[64:96], np.r_[32:64], np.r_[96:128]])
        cols = np.concatenate([m * 128 + P for m in range(w.shape[-1] // 128)])
        w = w[:, cols]
    return np.ascontiguousarray(w.reshape(-1, 128, w.shape[-1]).transpose(1, 0, 2))


def core_input_map(k, q, v, w_key, w_query, w_value, w_proj, core):
    import ml_dtypes

    bf16 = ml_dtypes.bfloat16
    b, g = core // 2, core % 2
    sl = slice(g * HGD, (g + 1) * HGD)
    wk_3 = _w_ktiled(np.asarray(w_key[:, sl]).astype(bf16), dr_perm=True)
    wq_3 = _w_ktiled(np.asarray(w_query[:, sl]).astype(bf16), dr_perm=True)
    wv_t = _w_ktiled(np.asarray(w_value[:, sl]).astype(bf16)).reshape(128, -1)
    return {
        "xq": np.ascontiguousarray(np.asarray(q[b]).astype(bf16)),
        "xk": np.ascontiguousarray(np.asarray(k[b]).astype(bf16)),
        "xv": np.ascontiguousarray(np.asarray(v[b]).astype(bf16)),
        "wkq": np.ascontiguousarray(
            np.concatenate(
                [
                    wk_3[:, :, 0:128].reshape(128, -1),
                    wq_3[:, :, 0:128].reshape(128, -1),
                    wk_3[:, :, 128:512].reshape(128, -1),
                    wq_3[:, :, 128:512].reshape(128, -1),
                ],
                axis=1,
            )
        ),
        "wvi": np.ascontiguousarray(
            np.concatenate([wv_t, np.eye(128, dtype=bf16)], axis=1)
        ),
        "wp": _w_ktiled(np.asarray(w_proj[sl, :]).astype(bf16)),
    }


def kernel(k, q, v, w_key, w_query, w_value, w_proj):
    from concourse.bass_utils import run_bass_kernel_spmd

    nc = _get_nc()
    in_maps = [
        core_input_map(k, q, v, w_key, w_query, w_value, w_proj, c) for c in range(8)
    ]
    res = run_bass_kernel_spmd(nc, in_maps, list(range(8))).results
    out = np.empty((B, T, EMB), np.float32)
    for b in range(B):
        out[b] = res[2 * b]["y"] + res[2 * b + 1]["y"]
    return out

